# revision 8
# baseline (speedup 1.0000x reference)
"""BitLinear forward on 8 Trainium2 NeuronCores.

Computation (reference):
    threshold = mean(|W|) * 0.7            (global scalar over full W)
    Wq = sign(W) * (|W| > threshold)       (ternary {-1, 0, 1})
    y = x @ (Wq * scale).T                 (x: [4, 2048, 4096], W: [11008, 4096])

Sharding: column-parallel over out_features. Each core owns a 1376-row slice
of W (1376 = 11008/8, no padding: per-matmul free dims are 512+512+352), gets
the full x, and computes its slice of the output. The global mean needs a
cross-core AllGather of one scalar.

Schedule (the PE runs at 13/16 of 2.4 GHz under the board's GPIO power
throttle, so the streaming roofline is ~1.44 ms; everything else is arranged
to keep the PE gap-free at that rate):

    T:  stream an f16 copy of W^T (half the HBM bytes of the f32 pass; the
        |w| mean from f16 values is ~1e-7 relative off, vs ~1.5e-4 of slack
        before any weight flips classification), |.|-reduce per partition,
        collapse+broadcast with a ones[128,128] matmul, AllGather the 8
        per-core [128,1] partials, strided-gather to [128,8], local sum ->
        threshold on all 128 partitions. All DMAs on the critical path are
        HWDGE (no SWDGE drains).
    Q1: ternarize cols [0:1024) from the f32 W^T (exact classification):
        cl = clamp(w, -t, t) and df = w - cl on DVE (df in bf16: exactly 0
        iff |w| <= t, sign preserved otherwise), wq = sign(df) on ScalarE
        written directly as fp8e4 (ternary is exact in fp8) into the
        resident Wq^T.
    E:  m-tiles 0..3 x chunks {0,1} (o < 1024) interleaved over k: 8 PSUM
        banks -> the PE consumes 4096 cols per wq k-tile (~2.1 us), just
        above the Q1 production rate, so the PE never starves while the
        ternarize pipeline finishes.
    Q2: ternarize cols [1024:1376) (clamp on DVE, subtract on GpSimd, sign
        on ScalarE) - runs in the engine headroom behind phase E.
    C:  catch-up chunk 2 for m-tiles 0..3, k-major across the 4 PSUM banks.
    S:  m-tiles 4..63, one at a time: 3 chunk matmuls per k accumulating in
        3 PSUM banks, scale on eviction, DMA out.

Matmul dtypes: stationary x in f16 (cast on host - identical rounding to a
device-side cast; ~2e-4 relative error on y), moving Wq in fp8e4 (exact),
fp32 PSUM accumulate. 1 col/cycle on the PE either way; fp8 halves the
resident Wq footprint (43 KB/partition).
"""

import numpy as np

import concourse.mybir as mybir
import concourse.tile as tile
from concourse import bacc
from concourse.bass_utils import run_bass_kernel_spmd
from concourse.tile import add_dep_helper

N_CORES = 8
O_FULL = 11008
K = 4096
M = 8192
O_SL = O_FULL // N_CORES  # 1376
KT = K // 128  # 32
MT = M // 128  # 64
O_CHUNKS = ((0, 512), (512, 512), (1024, 352))
O_SPLIT = 1024  # ternarize pass-1 covers [0:O_SPLIT), pass-2 the rest
W_COUNT = float(O_FULL) * float(K)
THRESH_FACTOR = 0.7

_nc_cache = {}


def _build(scale_one: bool):
    nc = bacc.Bacc(None, target_bir_lowering=False)
    f32 = mybir.dt.float32
    f16 = mybir.dt.float16
    bf16 = mybir.dt.bfloat16
    f8 = mybir.dt.float8e4

    # x pre-tiled and pre-cast on host: xt[mo, ki, ko, mi] = x[mo*128+mi, ko*128+ki]
    xt = nc.dram_tensor("xt", [MT, 128, KT, 128], f16, kind="ExternalInput")
    # W slice transposed: wt[i, o] = W[o_global, i]
    wt = nc.dram_tensor("wt", [K, O_SL], f32, kind="ExternalInput")
    # f16 copy of the same, for the |W|-mean pass only
    wt16 = nc.dram_tensor("wt16", [K, O_SL], f16, kind="ExternalInput")
    # scale slice replicated to 128 partitions on host
    sc = nc.dram_tensor("sc", [128, O_SL], f32, kind="ExternalInput")
    y = nc.dram_tensor("y", [M, O_SL], f32, kind="ExternalOutput")

    wt_t = wt[:].rearrange("(ko ki) o -> ki ko o", ki=128)  # [128, KT, O_SL]
    wt16_t = wt16[:].rearrange("(ko ki) o -> ki ko o", ki=128)

    with tile.TileContext(nc) as tc:
        with (
            tc.tile_pool(name="const", bufs=1) as const,
            tc.tile_pool(name="wldt", bufs=10) as wldt,
            tc.tile_pool(name="wld1", bufs=6) as wld1,
            tc.tile_pool(name="wld2", bufs=4) as wld2,
            tc.tile_pool(name="qtmp", bufs=2) as qtmp,
            tc.tile_pool(name="wq", bufs=1) as wqp,
            tc.tile_pool(name="xin", bufs=5) as xin,
            tc.tile_pool(name="yout", bufs=5) as yout,
            tc.tile_pool(name="mm_psum", bufs=8, space="PSUM") as mmps,
            tc.tile_pool(name="dram", bufs=1, space="DRAM") as dram,
        ):
            ones = const.tile([128, 128], f32)
            nc.any.memset(ones[:], 1.0)
            scale_sb = const.tile([128, O_SL], f32)
            sc_dma = nc.sync.dma_start(scale_sb[:], sc[:])

            # ---- phase T: partial sum of |W| on this core (f16 copy)
            # Each k-tile's |.|-row-sum is split across DVE (reduce_sum) and
            # ScalarE (activation Abs with accum_out) so the reduce keeps up
            # with the ~1 us/tile DMA cadence (a full-tile DVE reduce alone
            # is ~1.6 us/tile and was the T-pass critical path).
            T_SPL = 688
            acc = const.tile([128, KT], f32)
            acc2 = const.tile([128, KT], f32)
            last_t_dma = None
            for k in range(KT):
                w_k = wldt.tile([128, O_SL], f16, tag="wldt")
                last_t_dma = nc.sync.dma_start(w_k[:], wt16_t[:, k])
                nc.vector.reduce_sum(
                    acc[:, k : k + 1],
                    w_k[:, 0:T_SPL],
                    axis=mybir.AxisListType.X,
                    apply_absolute_value=True,
                )
                trash = qtmp.tile([128, O_SL - T_SPL], f16, tag="trash")
                nc.scalar.activation(
                    trash[:],
                    w_k[:, T_SPL:O_SL],
                    mybir.ActivationFunctionType.Abs,
                    accum_out=acc2[:, k : k + 1],
                )
            # the scale load is not needed until the first PSUM eviction
            add_dep_helper(sc_dma.ins, last_t_dma.ins, False, "scale after T pass")
            r1 = const.tile([128, 1], f32)
            nc.vector.reduce_sum(r1[:], acc[:], axis=mybir.AxisListType.X)
            r2 = const.tile([128, 1], f32)
            nc.vector.reduce_sum(r2[:], acc2[:], axis=mybir.AxisListType.X)
            red = const.tile([128, 1], f32)
            nc.vector.tensor_tensor(red[:], r1[:], r2[:], mybir.AluOpType.add)
            # collapse to one scalar: ones[:,0]^T(1x128) @ red(128x1) -> [1,1]
            ps_s = mmps.tile([1, 1], f32, tag="ps", name="ps_s")
            nc.tensor.matmul(
                ps_s[:], lhsT=ones[:, 0:1], rhs=red[:], start=True, stop=True
            )
            part = const.tile([1, 1], f32)
            nc.vector.tensor_copy(part[:], ps_s[:])

            # AllGather the 8 per-core scalars, then reduce + threshold math
            # on partition 0 and broadcast the result with a K=1 matmul -- no
            # SWDGE broadcast on the critical path.
            cin = dram.tile([1, 1], f32)
            cout = dram.tile([N_CORES, 1], f32, addr_space="Shared")
            nc.sync.dma_start(cin[:], part[:])
            nc.gpsimd.collective_compute(
                "AllGather",
                mybir.AluOpType.bypass,
                ins=[cin.opt()],
                outs=[cout.opt()],
                replica_groups=[list(range(N_CORES))],
            )
            gat = const.tile([1, N_CORES], f32)
            nc.sync.dma_start(gat[:], cout[:].rearrange("a b -> b a"))
            tot0 = const.tile([1, 1], f32)
            nc.vector.reduce_sum(tot0[:], gat[:], axis=mybir.AxisListType.X)
            thr0 = const.tile([1, 1], f32)
            nc.vector.tensor_scalar(
                thr0[:],
                tot0[:],
                float(np.float32(1.0) / np.float32(W_COUNT)),
                THRESH_FACTOR,
                mybir.AluOpType.mult,
                mybir.AluOpType.mult,
            )
            # broadcast to all 128 partitions: ones[0,:]^T(128x1) @ thr0(1x1)
            ps_b = mmps.tile([128, 1], f32, tag="ps", name="ps_b")
            nc.tensor.matmul(
                ps_b[:], lhsT=ones[0:1, :], rhs=thr0[:], start=True, stop=True
            )
            thr = const.tile([128, 1], f32)
            nc.vector.tensor_copy(thr[:], ps_b[:])
            nthr = const.tile([128, 1], f32)
            nc.vector.tensor_scalar_mul(nthr[:], thr[:], -1.0)

            # early x tiles for phases E and C (issued before the bulk W
            # re-read so they are on-chip by the time the threshold lands)
            e_mos = [0, 1, 2, 3]
            xts = {}
            for mo in e_mos:
                xt_sb = xin.tile([128, KT, 128], f16, tag="xt", name=f"xt_{mo}")
                x_dma = nc.sync.dma_start(xt_sb[:], xt[mo])
                add_dep_helper(x_dma.ins, last_t_dma.ins, False, "x after T pass")
                xts[mo] = xt_sb

            # ---- phase Q: ternarize into resident fp8 Wq^T
            # wq = sign(w - clamp(w, -t, t)): exactly 0 for |w| <= t, else
            # +-1. Two column passes: [0:1024) feeds phase E immediately;
            # [1024:1376) fills in behind it (subtract on GpSimd so DVE keeps
            # pace with the phase-E consumption rate).
            wq_sb = wqp.tile([128, KT, O_SL], f8)
            for k in range(KT):
                w_k = wld1.tile([128, O_SPLIT], f32, tag="wld1")
                q_dma = nc.sync.dma_start(w_k[:], wt_t[:, k, 0:O_SPLIT])
                add_dep_helper(q_dma.ins, last_t_dma.ins, False, "W re-read after T")
                # the first two k-tiles are produced in half-width pieces so
                # the PE can start on chunk 0 ~1.5 us earlier (pipeline ramp)
                pieces = ((0, 512), (512, 512)) if k < 2 else ((0, O_SPLIT),)
                for p0, pw in pieces:
                    cl = qtmp.tile([128, pw], f32, tag="cl", name=f"cl{k}_{p0}")
                    nc.vector.tensor_scalar(
                        cl[:], w_k[:, p0 : p0 + pw], thr[:], nthr[:],
                        mybir.AluOpType.min, mybir.AluOpType.max,
                    )
                    df = qtmp.tile([128, pw], bf16, tag="df", name=f"df{k}_{p0}")
                    nc.vector.tensor_tensor(
                        df[:], w_k[:, p0 : p0 + pw], cl[:], mybir.AluOpType.subtract
                    )
                    nc.scalar.sign(wq_sb[:, k, p0 : p0 + pw], df[:])

            o2w = O_SL - O_SPLIT  # 352
            for k in range(KT):
                w_k = wld2.tile([128, o2w], f32, tag="wld2")
                q_dma = nc.sync.dma_start(w_k[:], wt_t[:, k, O_SPLIT:O_SL])
                add_dep_helper(q_dma.ins, last_t_dma.ins, False, "W tail after T")
                cl = qtmp.tile([128, o2w], f32, tag="cl2")
                nc.vector.tensor_scalar(
                    cl[:], w_k[:], thr[:], nthr[:],
                    mybir.AluOpType.min, mybir.AluOpType.max,
                )
                df = qtmp.tile([128, o2w], bf16, tag="df2")
                # DVE (not GpSimd): phase C consumes a tail tile every ~720 ns
                # and the GpSimd 2-input subtract (~950 ns) was the limiter;
                # DVE finishes pass-1 with ~6 us to spare and stays ahead
                nc.vector.tensor_tensor(df[:], w_k[:], cl[:], mybir.AluOpType.subtract)
                nc.scalar.sign(wq_sb[:, k, O_SPLIT:O_SL], df[:])

            def evict(yr, ps_t, o0, w):
                if scale_one:
                    # scale == 1: plain copy on the otherwise-idle ScalarE
                    nc.scalar.copy(yr[:, o0 : o0 + w], ps_t[:, :w])
                else:
                    nc.vector.tensor_tensor(
                        yr[:, o0 : o0 + w],
                        ps_t[:, :w],
                        scale_sb[:, o0 : o0 + w],
                        mybir.AluOpType.mult,
                    )

            # ---- phase E: m-tiles 0..3, chunks {0,1}, k-interleaved (8 banks)
            ps_e = {
                (mo, ci): mmps.tile([128, 512], f32, tag="ps", name=f"pse{mo}_{ci}")
                for mo in e_mos
                for ci in (0, 1)
            }
            for k in range(KT):
                # chunk-outer order: the first 4 matmuls of each k need only
                # wq[:, k, 0:512], which the half-width ternarize pieces of
                # the first k-tiles produce first
                for ci in (0, 1):
                    o0, w = O_CHUNKS[ci]
                    for mo in e_mos:
                        nc.tensor.matmul(
                            ps_e[mo, ci][:, :w],
                            lhsT=xts[mo][:, k, :],
                            rhs=wq_sb[:, k, o0 : o0 + w],
                            start=(k == 0),
                            stop=(k == KT - 1),
                        )
            yrs = {}
            for mo in e_mos:
                yr = yout.tile([128, O_SL], f32, tag="yr", name=f"yr_{mo}")
                for ci in (0, 1):
                    o0, w = O_CHUNKS[ci]
                    evict(yr, ps_e[mo, ci], o0, w)
                yrs[mo] = yr

            # ---- phase C: catch-up chunk 2 for m-tiles 0..3, k-major
            o0c, wc = O_CHUNKS[2]
            ps_c = {
                mo: mmps.tile([128, wc], f32, tag="ps", name=f"psc{mo}")
                for mo in e_mos
            }
            for k in range(KT):
                for mo in e_mos:
                    nc.tensor.matmul(
                        ps_c[mo][:],
                        lhsT=xts[mo][:, k, :],
                        rhs=wq_sb[:, k, o0c : o0c + wc],
                        start=(k == 0),
                        stop=(k == KT - 1),
                    )
            for mo in e_mos:
                evict(yrs[mo], ps_c[mo], o0c, wc)
                nc.sync.dma_start(y[mo * 128 : (mo + 1) * 128, :], yrs[mo][:])

            # ---- phase S: remaining m-tiles, one at a time
            for mo in range(4, MT):
                xt_sb = xin.tile([128, KT, 128], f16, tag="xt", name=f"xt_{mo}")
                x_dma = nc.sync.dma_start(xt_sb[:], xt[mo])
                if mo < 6:
                    add_dep_helper(x_dma.ins, last_t_dma.ins, False, "x after T")
                ps3 = [
                    mmps.tile([128, 512], f32, tag="ps", name=f"ps{mo}_{ci}")
                    for ci in range(3)
                ]
                for k in range(KT):
                    for ci, (o0, w) in enumerate(O_CHUNKS):
                        nc.tensor.matmul(
                            ps3[ci][:, :w],
                            lhsT=xt_sb[:, k, :],
                            rhs=wq_sb[:, k, o0 : o0 + w],
                            start=(k == 0),
                            stop=(k == KT - 1),
                        )
                yr = yout.tile([128, O_SL], f32, tag="yr", name=f"yr_{mo}")
                for ci, (o0, w) in enumerate(O_CHUNKS):
                    evict(yr, ps3[ci], o0, w)
                nc.sync.dma_start(y[mo * 128 : (mo + 1) * 128, :], yr[:])

    nc.compile()
    return nc


def _get_nc(scale_one: bool):
    if scale_one not in _nc_cache:
        _nc_cache[scale_one] = _build(scale_one)
    return _nc_cache[scale_one]


def _prep_inputs(x: np.ndarray, weight: np.ndarray, scale: np.ndarray):
    xf = np.ascontiguousarray(x, dtype=np.float32).reshape(M, K)
    # xt[mo, ki, ko, mi] = x[mo*128+mi, ko*128+ki], cast to f16 on host (the
    # device would do the identical rounding before the f16 matmul anyway)
    xt = np.ascontiguousarray(
        xf.reshape(MT, 128, KT, 128).transpose(0, 3, 2, 1).astype(np.float16)
    )
    in_maps = []
    for c in range(N_CORES):
        wsl = weight[c * O_SL : (c + 1) * O_SL].astype(np.float32, copy=False)
        wt = np.ascontiguousarray(wsl.T)  # [K, O_SL]
        wt16 = np.ascontiguousarray(wt.astype(np.float16))
        ssl = scale[c * O_SL : (c + 1) * O_SL].astype(np.float32, copy=False)
        sc = np.ascontiguousarray(
            np.broadcast_to(ssl.reshape(1, O_SL), (128, O_SL))
        )
        in_maps.append({"xt": xt, "wt": wt, "wt16": wt16, "sc": sc})
    return in_maps


def _run(x, weight, scale, split_lo=None, x_raw=None, **run_kwargs):
    del split_lo, x_raw  # legacy knobs from the f32-input variant
    scale_one = bool(np.all(np.asarray(scale) == 1.0))
    nc = _get_nc(scale_one)
    in_maps = _prep_inputs(x, weight, scale)
    res = run_bass_kernel_spmd(nc, in_maps, core_ids=list(range(N_CORES)), **run_kwargs)
    parts = [res.results[c]["y"] for c in range(N_CORES)]
    y = np.concatenate(parts, axis=1).reshape(4, 2048, O_FULL).astype(np.float32)
    return y, res


def kernel(x: np.ndarray, weight: np.ndarray, scale: np.ndarray) -> np.ndarray:
    y, _ = _run(x, weight, scale)
    return y


# revision 9
# speedup vs baseline: 1.0068x; 1.0068x over previous
"""BitLinear forward on 8 Trainium2 NeuronCores.

Computation (reference):
    threshold = mean(|W|) * 0.7            (global scalar over full W)
    Wq = sign(W) * (|W| > threshold)       (ternary {-1, 0, 1})
    y = x @ (Wq * scale).T                 (x: [4, 2048, 4096], W: [11008, 4096])

Sharding: column-parallel over out_features. Each core owns a 1376-row slice
of W (1376 = 11008/8, no padding: per-matmul free dims are 512+512+352), gets
the full x, and computes its slice of the output. The global mean needs a
cross-core AllGather of one scalar.

Schedule (the PE runs at 13/16 of 2.4 GHz under the board's GPIO power
throttle, so the streaming roofline is ~1.44 ms; everything else is arranged
to keep the PE gap-free at that rate):

    T:  stream an f16 copy of W^T (half the HBM bytes of the f32 pass; the
        |w| mean from f16 values is ~1e-7 relative off, vs ~1.5e-4 of slack
        before any weight flips classification), |.|-reduce per partition,
        collapse+broadcast with a ones[128,128] matmul, AllGather the 8
        per-core [128,1] partials, strided-gather to [128,8], local sum ->
        threshold on all 128 partitions. All DMAs on the critical path are
        HWDGE (no SWDGE drains).
    Q1: ternarize cols [0:1024) from the f32 W^T (exact classification):
        cl = clamp(w, -t, t) and df = w - cl on DVE (df in bf16: exactly 0
        iff |w| <= t, sign preserved otherwise), wq = sign(df) on ScalarE
        written directly as fp8e4 (ternary is exact in fp8) into the
        resident Wq^T.
    E:  m-tiles 0..3 x chunks {0,1} (o < 1024) interleaved over k: 8 PSUM
        banks -> the PE consumes 4096 cols per wq k-tile (~2.1 us), just
        above the Q1 production rate, so the PE never starves while the
        ternarize pipeline finishes.
    Q2: ternarize cols [1024:1376) - runs on DVE/ScalarE after pass 1, in
        the engine headroom behind phase E, finishing before phase C needs
        the tail columns.
    C:  catch-up chunk 2 for m-tiles 0..3, k-major across the 4 PSUM banks.
    S:  m-tiles 4..63, one at a time: 3 chunk matmuls per k accumulating in
        3 PSUM banks, scale on eviction, DMA out.

Matmul dtypes: stationary x in f16 (cast on host - identical rounding to a
device-side cast; ~2e-4 relative error on y), moving Wq in fp8e4 (exact),
fp32 PSUM accumulate. 1 col/cycle on the PE either way; fp8 halves the
resident Wq footprint (43 KB/partition).
"""

import numpy as np

import concourse.mybir as mybir
import concourse.tile as tile
from concourse import bacc
from concourse.bass_utils import run_bass_kernel_spmd
from concourse.tile import add_dep_helper

N_CORES = 8
O_FULL = 11008
K = 4096
M = 8192
O_SL = O_FULL // N_CORES  # 1376
KT = K // 128  # 32
MT = M // 128  # 64
O_CHUNKS = ((0, 512), (512, 512), (1024, 352))
O_SPLIT = 1024  # ternarize pass-1 covers [0:O_SPLIT), pass-2 the rest
W_COUNT = float(O_FULL) * float(K)
THRESH_FACTOR = 0.7

_nc_cache = {}


def _build(scale_one: bool):
    nc = bacc.Bacc(None, target_bir_lowering=False)
    f32 = mybir.dt.float32
    f16 = mybir.dt.float16
    bf16 = mybir.dt.bfloat16
    f8 = mybir.dt.float8e4

    # x pre-tiled and pre-cast on host: xt[mo, ki, ko, mi] = x[mo*128+mi, ko*128+ki]
    xt = nc.dram_tensor("xt", [MT, 128, KT, 128], f16, kind="ExternalInput")
    # W slice transposed: wt[i, o] = W[o_global, i]
    wt = nc.dram_tensor("wt", [K, O_SL], f32, kind="ExternalInput")
    # f16 copy of the same, for the |W|-mean pass only
    wt16 = nc.dram_tensor("wt16", [K, O_SL], f16, kind="ExternalInput")
    # scale slice replicated to 128 partitions on host
    sc = nc.dram_tensor("sc", [128, O_SL], f32, kind="ExternalInput")
    y = nc.dram_tensor("y", [M, O_SL], f32, kind="ExternalOutput")

    wt_t = wt[:].rearrange("(ko ki) o -> ki ko o", ki=128)  # [128, KT, O_SL]
    wt16_t = wt16[:].rearrange("(ko ki) o -> ki ko o", ki=128)

    with tile.TileContext(nc) as tc:
        with (
            tc.tile_pool(name="const", bufs=1) as const,
            tc.tile_pool(name="wldt", bufs=10) as wldt,
            tc.tile_pool(name="wld1", bufs=6) as wld1,
            tc.tile_pool(name="wld2", bufs=4) as wld2,
            tc.tile_pool(name="qtmp", bufs=2) as qtmp,
            tc.tile_pool(name="wq", bufs=1) as wqp,
            tc.tile_pool(name="xin", bufs=5) as xin,
            tc.tile_pool(name="yout", bufs=5) as yout,
            tc.tile_pool(name="mm_psum", bufs=8, space="PSUM") as mmps,
            tc.tile_pool(name="dram", bufs=1, space="DRAM") as dram,
        ):
            ones = const.tile([128, 128], f32)
            nc.any.memset(ones[:], 1.0)
            scale_sb = const.tile([128, O_SL], f32)
            sc_dma = nc.sync.dma_start(scale_sb[:], sc[:])

            # ---- phase T: partial sum of |W| on this core (f16 copy)
            # Each k-tile's |.|-row-sum is split across DVE (reduce_sum) and
            # ScalarE (activation Abs with accum_out) so the reduce keeps up
            # with the ~1 us/tile DMA cadence (a full-tile DVE reduce alone
            # is ~1.6 us/tile and was the T-pass critical path).
            T_SPL = 688
            acc = const.tile([128, KT], f32)
            acc2 = const.tile([128, KT], f32)
            last_t_dma = None
            for k in range(KT):
                w_k = wldt.tile([128, O_SL], f16, tag="wldt")
                last_t_dma = nc.sync.dma_start(w_k[:], wt16_t[:, k])
                nc.vector.reduce_sum(
                    acc[:, k : k + 1],
                    w_k[:, 0:T_SPL],
                    axis=mybir.AxisListType.X,
                    apply_absolute_value=True,
                )
                trash = qtmp.tile([128, O_SL - T_SPL], f16, tag="trash")
                nc.scalar.activation(
                    trash[:],
                    w_k[:, T_SPL:O_SL],
                    mybir.ActivationFunctionType.Abs,
                    accum_out=acc2[:, k : k + 1],
                )
            # the scale load is not needed until the first PSUM eviction
            add_dep_helper(sc_dma.ins, last_t_dma.ins, False, "scale after T pass")
            r1 = const.tile([128, 1], f32)
            nc.vector.reduce_sum(r1[:], acc[:], axis=mybir.AxisListType.X)
            r2 = const.tile([128, 1], f32)
            nc.vector.reduce_sum(r2[:], acc2[:], axis=mybir.AxisListType.X)
            red = const.tile([128, 1], f32)
            nc.vector.tensor_tensor(red[:], r1[:], r2[:], mybir.AluOpType.add)
            # collapse to one scalar: ones[:,0]^T(1x128) @ red(128x1) -> [1,1]
            ps_s = mmps.tile([1, 1], f32, tag="ps", name="ps_s")
            nc.tensor.matmul(
                ps_s[:], lhsT=ones[:, 0:1], rhs=red[:], start=True, stop=True
            )
            part = const.tile([1, 1], f32)
            nc.vector.tensor_copy(part[:], ps_s[:])

            # AllGather the 8 per-core scalars, then reduce + threshold math
            # on partition 0 and broadcast the result with a K=1 matmul -- no
            # SWDGE broadcast on the critical path.
            cin = dram.tile([1, 1], f32)
            cout = dram.tile([N_CORES, 1], f32, addr_space="Shared")
            nc.sync.dma_start(cin[:], part[:])
            nc.gpsimd.collective_compute(
                "AllGather",
                mybir.AluOpType.bypass,
                ins=[cin.opt()],
                outs=[cout.opt()],
                replica_groups=[list(range(N_CORES))],
            )
            gat = const.tile([1, N_CORES], f32)
            nc.sync.dma_start(gat[:], cout[:].rearrange("a b -> b a"))
            tot0 = const.tile([1, 1], f32)
            nc.vector.reduce_sum(tot0[:], gat[:], axis=mybir.AxisListType.X)
            thr0 = const.tile([1, 1], f32)
            nc.vector.tensor_scalar(
                thr0[:],
                tot0[:],
                float(np.float32(1.0) / np.float32(W_COUNT)),
                THRESH_FACTOR,
                mybir.AluOpType.mult,
                mybir.AluOpType.mult,
            )
            # broadcast to all 128 partitions: ones[0,:]^T(128x1) @ thr0(1x1)
            ps_b = mmps.tile([128, 1], f32, tag="ps", name="ps_b")
            nc.tensor.matmul(
                ps_b[:], lhsT=ones[0:1, :], rhs=thr0[:], start=True, stop=True
            )
            thr = const.tile([128, 1], f32)
            nc.vector.tensor_copy(thr[:], ps_b[:])
            nthr = const.tile([128, 1], f32)
            nc.vector.tensor_scalar_mul(nthr[:], thr[:], -1.0)

            # early x tiles for phases E and C (issued before the bulk W
            # re-read so they are on-chip by the time the threshold lands)
            e_mos = [0, 1, 2, 3]
            xts = {}
            for mo in e_mos:
                xt_sb = xin.tile([128, KT, 128], f16, tag="xt", name=f"xt_{mo}")
                x_dma = nc.sync.dma_start(xt_sb[:], xt[mo])
                add_dep_helper(x_dma.ins, last_t_dma.ins, False, "x after T pass")
                xts[mo] = xt_sb

            # ---- phase Q: ternarize into resident fp8 Wq^T
            # wq = sign(w - clamp(w, -t, t)): exactly 0 for |w| <= t, else
            # +-1. Two column passes: [0:1024) feeds phase E immediately;
            # [1024:1376) fills in behind it (subtract on GpSimd so DVE keeps
            # pace with the phase-E consumption rate).
            wq_sb = wqp.tile([128, KT, O_SL], f8)
            for k in range(KT):
                w_k = wld1.tile([128, O_SPLIT], f32, tag="wld1")
                q_dma = nc.sync.dma_start(w_k[:], wt_t[:, k, 0:O_SPLIT])
                add_dep_helper(q_dma.ins, last_t_dma.ins, False, "W re-read after T")
                # the first two k-tiles are produced in half-width pieces so
                # the PE can start on chunk 0 ~1.5 us earlier (pipeline ramp)
                pieces = ((0, 512), (512, 512)) if k < 2 else ((0, O_SPLIT),)
                for p0, pw in pieces:
                    cl = qtmp.tile([128, pw], f32, tag="cl", name=f"cl{k}_{p0}")
                    nc.vector.tensor_scalar(
                        cl[:], w_k[:, p0 : p0 + pw], thr[:], nthr[:],
                        mybir.AluOpType.min, mybir.AluOpType.max,
                    )
                    df = qtmp.tile([128, pw], bf16, tag="df", name=f"df{k}_{p0}")
                    nc.vector.tensor_tensor(
                        df[:], w_k[:, p0 : p0 + pw], cl[:], mybir.AluOpType.subtract
                    )
                    nc.scalar.sign(wq_sb[:, k, p0 : p0 + pw], df[:])

            o2w = O_SL - O_SPLIT  # 352
            for k in range(KT):
                w_k = wld2.tile([128, o2w], f32, tag="wld2")
                q_dma = nc.sync.dma_start(w_k[:], wt_t[:, k, O_SPLIT:O_SL])
                add_dep_helper(q_dma.ins, last_t_dma.ins, False, "W tail after T")
                cl = qtmp.tile([128, o2w], f32, tag="cl2")
                nc.vector.tensor_scalar(
                    cl[:], w_k[:], thr[:], nthr[:],
                    mybir.AluOpType.min, mybir.AluOpType.max,
                )
                df = qtmp.tile([128, o2w], bf16, tag="df2")
                # DVE (not GpSimd): phase C consumes a tail tile every ~720 ns
                # and the GpSimd 2-input subtract (~950 ns) was the limiter;
                # DVE finishes pass-1 with ~6 us to spare and stays ahead
                nc.vector.tensor_tensor(df[:], w_k[:], cl[:], mybir.AluOpType.subtract)
                nc.scalar.sign(wq_sb[:, k, O_SPLIT:O_SL], df[:])

            def evict(yr, ps_t, o0, w):
                if scale_one:
                    # scale == 1: plain copy on the otherwise-idle ScalarE
                    nc.scalar.copy(yr[:, o0 : o0 + w], ps_t[:, :w])
                else:
                    nc.vector.tensor_tensor(
                        yr[:, o0 : o0 + w],
                        ps_t[:, :w],
                        scale_sb[:, o0 : o0 + w],
                        mybir.AluOpType.mult,
                    )

            # ---- phase E: m-tiles 0..3, chunks {0,1}, k-interleaved (8 banks)
            ps_e = {
                (mo, ci): mmps.tile([128, 512], f32, tag="ps", name=f"pse{mo}_{ci}")
                for mo in e_mos
                for ci in (0, 1)
            }
            for k in range(KT):
                # chunk-outer order: the first 4 matmuls of each k need only
                # wq[:, k, 0:512], which the half-width ternarize pieces of
                # the first k-tiles produce first
                for ci in (0, 1):
                    o0, w = O_CHUNKS[ci]
                    for mo in e_mos:
                        nc.tensor.matmul(
                            ps_e[mo, ci][:, :w],
                            lhsT=xts[mo][:, k, :],
                            rhs=wq_sb[:, k, o0 : o0 + w],
                            start=(k == 0),
                            stop=(k == KT - 1),
                        )
            yrs = {}
            for mo in e_mos:
                yr = yout.tile([128, O_SL], f32, tag="yr", name=f"yr_{mo}")
                for ci in (0, 1):
                    o0, w = O_CHUNKS[ci]
                    evict(yr, ps_e[mo, ci], o0, w)
                yrs[mo] = yr

            # ---- phase C: catch-up chunk 2 for m-tiles 0..3, k-major
            o0c, wc = O_CHUNKS[2]
            ps_c = {
                mo: mmps.tile([128, wc], f32, tag="ps", name=f"psc{mo}")
                for mo in e_mos
            }
            for k in range(KT):
                for mo in e_mos:
                    nc.tensor.matmul(
                        ps_c[mo][:],
                        lhsT=xts[mo][:, k, :],
                        rhs=wq_sb[:, k, o0c : o0c + wc],
                        start=(k == 0),
                        stop=(k == KT - 1),
                    )
            for mo in e_mos:
                evict(yrs[mo], ps_c[mo], o0c, wc)
                nc.sync.dma_start(y[mo * 128 : (mo + 1) * 128, :], yrs[mo][:])

            # ---- phase S: remaining m-tiles, one at a time
            for mo in range(4, MT):
                xt_sb = xin.tile([128, KT, 128], f16, tag="xt", name=f"xt_{mo}")
                x_dma = nc.sync.dma_start(xt_sb[:], xt[mo])
                if mo < 6:
                    add_dep_helper(x_dma.ins, last_t_dma.ins, False, "x after T")
                ps3 = [
                    mmps.tile([128, 512], f32, tag="ps", name=f"ps{mo}_{ci}")
                    for ci in range(3)
                ]
                for k in range(KT):
                    for ci, (o0, w) in enumerate(O_CHUNKS):
                        nc.tensor.matmul(
                            ps3[ci][:, :w],
                            lhsT=xt_sb[:, k, :],
                            rhs=wq_sb[:, k, o0 : o0 + w],
                            start=(k == 0),
                            stop=(k == KT - 1),
                        )
                yr = yout.tile([128, O_SL], f32, tag="yr", name=f"yr_{mo}")
                for ci, (o0, w) in enumerate(O_CHUNKS):
                    evict(yr, ps3[ci], o0, w)
                nc.sync.dma_start(y[mo * 128 : (mo + 1) * 128, :], yr[:])

    nc.compile()
    return nc


def _get_nc(scale_one: bool):
    if scale_one not in _nc_cache:
        _nc_cache[scale_one] = _build(scale_one)
    return _nc_cache[scale_one]


def _prep_inputs(x: np.ndarray, weight: np.ndarray, scale: np.ndarray):
    xf = np.ascontiguousarray(x, dtype=np.float32).reshape(M, K)
    # xt[mo, ki, ko, mi] = x[mo*128+mi, ko*128+ki], cast to f16 on host (the
    # device would do the identical rounding before the f16 matmul anyway)
    xt = np.ascontiguousarray(
        xf.reshape(MT, 128, KT, 128).transpose(0, 3, 2, 1).astype(np.float16)
    )
    in_maps = []
    for c in range(N_CORES):
        wsl = weight[c * O_SL : (c + 1) * O_SL].astype(np.float32, copy=False)
        wt = np.ascontiguousarray(wsl.T)  # [K, O_SL]
        wt16 = np.ascontiguousarray(wt.astype(np.float16))
        ssl = scale[c * O_SL : (c + 1) * O_SL].astype(np.float32, copy=False)
        sc = np.ascontiguousarray(
            np.broadcast_to(ssl.reshape(1, O_SL), (128, O_SL))
        )
        in_maps.append({"xt": xt, "wt": wt, "wt16": wt16, "sc": sc})
    return in_maps


def _run(x, weight, scale, split_lo=None, x_raw=None, **run_kwargs):
    del split_lo, x_raw  # legacy knobs from the f32-input variant
    scale_one = bool(np.all(np.asarray(scale) == 1.0))
    nc = _get_nc(scale_one)
    in_maps = _prep_inputs(x, weight, scale)
    res = run_bass_kernel_spmd(nc, in_maps, core_ids=list(range(N_CORES)), **run_kwargs)
    parts = [res.results[c]["y"] for c in range(N_CORES)]
    y = np.concatenate(parts, axis=1).reshape(4, 2048, O_FULL).astype(np.float32)
    return y, res


def kernel(x: np.ndarray, weight: np.ndarray, scale: np.ndarray) -> np.ndarray:
    y, _ = _run(x, weight, scale)
    return y
